# revision 43
# baseline (speedup 1.0000x reference)
"""Trainium2 Bass kernel for 3D conv-attention layer (v5).

Reference (per (b,h,w) "site", D=32 positions, S=32 features):
  k,q,v = 1x1 conv of x [B,C,D,H,W] -> [B,S,D,H,W]
  scoresT[j,i] = sum_s q[s,j] k[s,i] / sqrt(S)   (per site)
  aT = softmax over i  (free dim of scoresT)
  o[s,j] = sum_i v[s,i] a[i,j];   y = x + Wo @ o + bo

Sharding: data-parallel over H across 8 cores (HS=8 rows each).

v5.7 (185.3us) changes over v4 (224.5us):
  - Chunks are (b, h-quad): x loaded bf16 directly by a gpsimd software-DGE
    cast DMA from DRAM (1KB runs), halving input DMA bytes and removing the
    separate SBUF->SBUF cast. 8 chunks x 8 groups of 32 sites.
  - Out-projection: 2 matmuls of M=128 with block-structured lhsT (two
    r-groups' Wo stacked per matmul, K=128 full PE mode) instead of 8 M=32
    matmuls: 512 rows/group vs 2048. The y-add reads the op tile at partition
    base 0/64 (PSUM in0 with shifted base is legal; SBUF-SBUF is not).
  - softmax scale (e*rcp) on GPSIMD/Pool instead of DVE.
  - residual add reads the bf16 x directly (one consistent bf16 rounding for
    projections and residual); the bo output bias is added on the host in
    gather() - it is a per-channel constant independent of the device math.
  - x/y tiles are h-pair-major [C, 2, D, 2, W] so loads and stores split into
    half-chunk DMAs with full-rate runs (early pipeline fill, overlapped
    drain); constants ride 3 consolidated DMAs (HWDGE holds 625ns per DMA,
    8 serial constant DMAs used to delay the first projection to 6.3us).
  - softmax bookkeeping (den/rcp/e*rcp/transposes of v and a) runs on paired
    [128,512] SBUF tiles spanning two 32-site groups: the PSUM 256-col matmul
    cap does not apply to SBUF tiles that ACT copies fill in halves, so DVE
    instruction count and per-op access latency are halved for these stages.
  - the two out-proj matmuls write one [128,512] 1-bank PSUM tile (the v4
    ">=1024B free offset" landmine is refuted by direct probe - in-bank
    offsets are fine), so the residual is 2 double-width DVE adds, not 4.
  - the first and last group-pairs run their softmax bookkeeping per-group
    (unpaired): pairing adds a slot of chain latency, which is hidden mid-
    stream but directly lengthens pipeline fill and drain.
  - scores also pair: two groups' score matmuls write one [128,512] 1-bank
    PSUM tile (in-bank offsets >=1024B are legal - the v4 "offset landmine"
    is refuted by direct probe), so exp is one ACT op per pair. The freed
    PSUM bank double-buffers the out-proj tile (op bufs=2), decoupling each
    group's out-projection from the previous group's residual adds (-6us).
  HW landmines respected:
  no 32x64 PE tiles; multiple tile ROWS must never write the same PSUM
  partition range (crashes device) - all matmuls writing one tile use
  distinct cols (diagonal or (0,32r) placements only).
"""

import math
from contextlib import ExitStack

import numpy as np

import concourse.bass as bass
import concourse.mybir as mybir
from concourse import bacc
import concourse.tile as tile
from concourse.bass_utils import run_bass_kernel_spmd

B, C, D, H, W = 4, 64, 32, 64, 64
S = C // 2  # 32
NCORES = 8
HS = H // NCORES  # 8 h-rows per core
NCH = 2           # chunks per b: h-quads
HQ = 4            # h rows per chunk
NG = 8            # site-groups (32 sites) per chunk
FW = 2 * W        # 128: free stride of d within an h-pair block
F32 = mybir.dt.float32
BF16 = mybir.dt.bfloat16

INV_SQRT_S = 1.0 / math.sqrt(S)


def mkap(base, part0, pcount, foff, fdims):
    """AP at partition block [part0, part0+pcount) of a tile, free offset foff,
    free dims [(step, count), ...] in the tile's flat free space."""
    full = base[...] if not isinstance(base, bass.AP) else base
    pstride = full.ap[0][0]
    return bass.AP(tensor=full.tensor,
                   offset=full.offset + part0 * pstride + foff,
                   ap=[[pstride, pcount]] + [list(d) for d in fdims])


def build_program():
    nc = bacc.Bacc()
    x_d = nc.declare_dram_parameter("x", [B, C, D, HS, W], F32, isOutput=False)
    wkqv_d = nc.declare_dram_parameter("wkqvT", [C, 3 * S], BF16, isOutput=False)
    wo_d = nc.declare_dram_parameter("woPP", [128, 256], BF16, isOutput=False)
    br_d = nc.declare_dram_parameter("brs", [128, 3], F32, isOutput=False)
    y_d = nc.declare_dram_parameter("y", [B, C, D, HS, W], F32, isOutput=True)

    CH = [(b, hq) for b in range(B) for hq in range(NCH)]
    NT = len(CH) * NG  # 64 groups

    with tile.TileContext(nc) as tc, ExitStack() as ctx:
        const = ctx.enter_context(tc.tile_pool(name="const", bufs=1))
        xp = ctx.enter_context(tc.tile_pool(name="xp", bufs=3))
        yp = ctx.enter_context(tc.tile_pool(name="yp", bufs=2))
        kg_ps = ctx.enter_context(tc.tile_pool(name="kg_ps", bufs=1, space="PSUM"))
        qg_ps = ctx.enter_context(tc.tile_pool(name="qg_ps", bufs=1, space="PSUM"))
        vg_ps = ctx.enter_context(tc.tile_pool(name="vg_ps", bufs=1, space="PSUM"))
        sc_ps = ctx.enter_context(tc.tile_pool(name="sc_ps", bufs=2, space="PSUM"))
        og_ps = ctx.enter_context(tc.tile_pool(name="og_ps", bufs=1, space="PSUM"))
        op_ps = ctx.enter_context(tc.tile_pool(name="op_ps", bufs=2, space="PSUM"))
        sb = ctx.enter_context(tc.tile_pool(name="sb", bufs=2))

        # ---- first chunk load (emitted before constants so the DMA starts
        # immediately) ----
        chunks = {}

        def emit_load_half(ci, hp):
            b, hq = CH[ci]
            h0 = HQ * hq
            if hp == 0:
                x_bf = xp.tile([C, 2, D, 2, W], BF16, tag="xbf")
                chunks[ci] = {"xbf": x_bf}
            x_bf = chunks[ci]["xbf"]
            nc.gpsimd.dma_start(
                out=x_bf[:, hp, :, :, :],
                in_=x_d[b, :, :, h0 + 2 * hp:h0 + 2 * hp + 2, :])

        def emit_load(ci):
            emit_load_half(ci, 0)
            emit_load_half(ci, 1)

        emit_load(0)

        # ---- constants ----
        wkqv = const.tile([C, 3 * S], BF16, tag="wkqv")
        nc.sync.dma_start(out=wkqv[:, :], in_=wkqv_d[:, :])
        wkT, wqT, wvT = (wkqv[:, S * i:S * i + S] for i in range(3))
        woP = const.tile([128, 2, 128], BF16, tag="wo")
        nc.sync.dma_start(out=woP[:, :, :], in_=wo_d[:, :])
        brs = const.tile([128, 3], F32, tag="brs")
        nc.sync.dma_start(out=brs[:, :], in_=br_d[:, :])
        bk_t, bq_t, bv_t = (brs[:, i:i + 1] for i in range(3))

        def emit_ytile(ci):
            # residual comes straight from the bf16 x; bo is added on the host
            st = chunks[ci]
            y_sb = yp.tile([C, 2, D, 2, W], F32, tag="y")
            st["y"] = y_sb

        def goff(g):
            # x/y tiles are [C, 2(hpair), D, 2(hh), W]
            return 4096 * (g >> 2) + 64 * ((g >> 1) & 1) + 32 * (g & 1)

        def xap(base, g, r):
            # AP over (u:8, d:32) site columns of r-group r in group g
            return mkap(base, 0, C, goff(g) + r, [[4, 8], [FW, D]])

        pairs = {}

        def s1_proj_scores(t):
            ci, g = divmod(t, NG)
            st = chunks[ci]
            gs = st.setdefault(g, {})
            pi, ph = divmod(t, 2)
            if ph == 0:
                v_pair = sb.tile([128, 512], BF16, tag="vp", bufs=3)
                e_pair = sb.tile([128, 512], BF16, tag="ep", bufs=3)
                pairs[pi] = {"v": v_pair, "e": e_pair}
            pr = pairs[pi]
            kg = kg_ps.tile([128, 256], F32, tag="kg")
            qg = qg_ps.tile([128, 256], F32, tag="qg")
            vg = vg_ps.tile([128, 256], F32, tag="vg")
            for r in range(4):
                rhs = xap(st["xbf"], g, r)
                nc.tensor.matmul(kg[32 * r:32 * r + 32, :], wkT, rhs,
                                 start=True, stop=True, tile_position=(0, 32 * r))
                nc.tensor.matmul(qg[32 * r:32 * r + 32, :], wqT, rhs,
                                 start=True, stop=True, tile_position=(0, 32 * r))
                nc.tensor.matmul(vg[32 * r:32 * r + 32, :], wvT, rhs,
                                 start=True, stop=True, tile_position=(0, 32 * r))
            k_sb = sb.tile([128, 256], BF16, tag="k", bufs=3)
            q_sb = sb.tile([128, 256], BF16, tag="q", bufs=3)
            v_sb = mkap(pr["v"], 0, 128, 256 * ph, [[1, 256]])
            nc.scalar.activation(k_sb[:, :], kg[:, :],
                                 mybir.ActivationFunctionType.Identity,
                                 bias=bk_t)
            nc.scalar.activation(q_sb[:, :], qg[:, :],
                                 mybir.ActivationFunctionType.Identity,
                                 bias=bq_t)
            nc.scalar.activation(v_sb, vg[:, :],
                                 mybir.ActivationFunctionType.Identity,
                                 bias=bv_t)
            sc = sc_ps.tile([128, 256], F32, tag="sc")
            for u in range(8):
                for r in range(4):
                    nc.tensor.matmul(
                        sc[32 * r:32 * r + 32, 32 * u:32 * u + 32],
                        q_sb[32 * r:32 * r + 32, 32 * u:32 * u + 32],
                        k_sb[32 * r:32 * r + 32, 32 * u:32 * u + 32],
                        start=True, stop=True,
                        tile_position=(32 * r, 32 * r))
            e_sb = mkap(pr["e"], 0, 128, 256 * ph, [[1, 256]])
            nc.scalar.activation(e_sb, sc[:, :],
                                 mybir.ActivationFunctionType.Exp,
                                 scale=INV_SQRT_S)
            if pi in (0, NT // 2 - 1):
                if ph == 0:
                    vT_pair = sb.tile([128, 512], BF16, tag="vTp", bufs=3)
                    pr["vT"] = vT_pair
                nc.vector.transpose(
                    mkap(pr["vT"], 0, 128, 256 * ph, [[1, 256]]),
                    mkap(pr["v"], 0, 128, 256 * ph, [[1, 256]]))
            elif ph == 1:
                vT_pair = sb.tile([128, 512], BF16, tag="vTp", bufs=3)
                nc.vector.transpose(vT_pair[:, :], pr["v"][:, :])
                pr["vT"] = vT_pair

        def s2a_softmax(t):
            pi, ph = divmod(t, 2)
            if pi in (0, NT // 2 - 1):
                pr = pairs[pi]
                den = sb.tile([128, 8], F32, tag="den1")
                nc.vector.reduce_sum(
                    out=den[:, :],
                    in_=mkap(pr["e"], 0, 128, 256 * ph, [[32, 8], [1, 32]]),
                    axis=mybir.AxisListType.X)
                rcp = sb.tile([128, 8], F32, tag="rcp1")
                nc.vector.reciprocal(rcp[:, :], den[:, :])
                if ph == 0:
                    aTp1 = sb.tile([128, 512], BF16, tag="aTp", bufs=2)
                    pr["aT"] = aTp1
                nc.gpsimd.tensor_tensor(
                    out=mkap(pr["aT"], 0, 128, 256 * ph, [[32, 8], [1, 32]]),
                    in0=mkap(pr["e"], 0, 128, 256 * ph, [[32, 8], [1, 32]]),
                    in1=mkap(rcp, 0, 128, 0, [[1, 8], [0, 32]]),
                    op=mybir.AluOpType.mult)
                return
            if t % 2 == 0:
                return
            pr = pairs[t // 2]
            e_pair = pr["e"]
            den = sb.tile([128, 16], F32, tag="den")
            nc.vector.reduce_sum(
                out=den[:, :],
                in_=mkap(e_pair, 0, 128, 0, [[32, 16], [1, 32]]),
                axis=mybir.AxisListType.X)
            rcp = sb.tile([128, 16], F32, tag="rcp")
            nc.vector.reciprocal(rcp[:, :], den[:, :])
            aT_pair = sb.tile([128, 512], BF16, tag="aTp", bufs=2)
            # e * rcp (broadcast over i) on Pool, whole pair at once
            nc.gpsimd.tensor_tensor(
                out=mkap(aT_pair, 0, 128, 0, [[32, 16], [1, 32]]),
                in0=mkap(e_pair, 0, 128, 0, [[32, 16], [1, 32]]),
                in1=mkap(rcp, 0, 128, 0, [[1, 16], [0, 32]]),
                op=mybir.AluOpType.mult)
            pr["aT"] = aT_pair

        def s2b_atrans(t):
            pi, ph = divmod(t, 2)
            if pi in (0, NT // 2 - 1):
                pr = pairs[pi]
                if ph == 0:
                    ap1 = sb.tile([128, 512], BF16, tag="ap", bufs=3)
                    pr["a"] = ap1
                nc.vector.transpose(
                    mkap(pr["a"], 0, 128, 256 * ph, [[1, 256]]),
                    mkap(pr["aT"], 0, 128, 256 * ph, [[1, 256]]))
                if ph == 1:
                    pr.pop("aT")
                return
            if t % 2 == 0:
                return
            pr = pairs[t // 2]
            a_pair = sb.tile([128, 512], BF16, tag="ap", bufs=3)
            nc.vector.transpose(a_pair[:, :], pr.pop("aT")[:, :])
            pr["a"] = a_pair

        def s3_att(t):
            ci, g = divmod(t, NG)
            gs = chunks[ci][g]
            pi, ph = divmod(t, 2)
            pr = pairs[pi]
            a_sb, vT_sb = pr["a"], pr["vT"]
            co = 256 * ph
            og = og_ps.tile([128, 256], F32, tag="og")
            for u in range(8):
                for r in range(4):
                    nc.tensor.matmul(
                        og[32 * r:32 * r + 32, 32 * u:32 * u + 32],
                        vT_sb[32 * r:32 * r + 32, co + 32 * u:co + 32 * u + 32],
                        a_sb[32 * r:32 * r + 32, co + 32 * u:co + 32 * u + 32],
                        start=True, stop=True,
                        tile_position=(32 * r, 32 * r))
            if ph == 1:
                pairs.pop(pi)
            o_sb = sb.tile([128, 256], BF16, tag="osb", bufs=3)
            nc.scalar.activation(o_sb[:, :], og[:, :],
                                 mybir.ActivationFunctionType.Copy)
            gs["o"] = o_sb

        def s4_outproj(t):
            ci, g = divmod(t, NG)
            st = chunks[ci]
            gs = st[g]
            o_sb = gs.pop("o")
            # one [128,512] PSUM tile (2 banks); the 2nd matmul writes at byte
            # offset 1024 = bank-2 start, which is legal (the v4 landmine is
            # within-bank offsets only). Half A = r0/r1, half B = r2/r3.
            opt = op_ps.tile([128, 512], F32, tag="op")
            for p in range(2):
                nc.tensor.matmul(mkap(opt, 0, 128, 256 * p, [[1, 256]]),
                                 woP[:, p, :], o_sb[:, :],
                                 start=True, stop=True, tile_position=(0, 0))
            # two double-width y-adds: parts 0-63 = (r0, r2), 64-127 = (r1, r3)
            for q in range(2):
                in0 = mkap(opt, 64 * q, C, 0, [[256, 2], [32, 8], [1, 32]])
                x_in = mkap(st["xbf"], 0, C, goff(g) + q,
                            [[2, 2], [4, 8], [FW, D]])
                y_out = mkap(st["y"], 0, C, goff(g) + q,
                             [[2, 2], [4, 8], [FW, D]])
                nc.vector.tensor_tensor(out=y_out, in0=in0, in1=x_in,
                                        op=mybir.AluOpType.add)
            gs.clear()

        def emit_store(ci, half):
            b, hq = CH[ci]
            h0 = HQ * hq + 2 * half
            nc.sync.dma_start(out=y_d[b, :, :, h0:h0 + 2, :],
                              in_=chunks[ci]["y"][:, half, :, :, :])

        import os
        K1, K2, K3 = (int(v) for v in os.environ.get("V5_SKEW", "1,4,6").split(","))
        ORDER = os.environ.get("V5_ORDER", "2a,4,1,3,2b").split(",")

        def emit_slot(t):
            for stage in ORDER:
                if stage == "1" and t < NT:
                    ci, g = divmod(t, NG)
                    if g == 0:
                        emit_ytile(ci)
                    if g in (4, 6) and ci + 1 < len(CH):
                        emit_load_half(ci + 1, (g - 4) // 2)
                    s1_proj_scores(t)
                elif stage == "2a" and 0 <= t - K1 < NT:
                    s2a_softmax(t - K1)
                elif stage == "2b" and 0 <= t - K1 < NT:
                    s2b_atrans(t - K1)
                elif stage == "3" and 0 <= t - K2 < NT:
                    s3_att(t - K2)
                elif stage == "4" and 0 <= t - K3 < NT:
                    s4_outproj(t - K3)
                    ci, g = divmod(t - K3, NG)
                    if g == NG // 2 - 1:
                        emit_store(ci, 0)
                    elif g == NG - 1:
                        emit_store(ci, 1)
                        del chunks[ci]

        for t in range(NT + K3 + 1):
            emit_slot(t)

    nc.finalize()
    return nc


_NC_CACHE = {}


def get_nc(key="v5"):
    if key not in _NC_CACHE:
        _NC_CACHE[key] = build_program()
    return _NC_CACHE[key]


def make_in_maps(x, Wk, bk, Wq, bq, Wv, bv, Wo, bo):
    import ml_dtypes
    x = np.ascontiguousarray(np.asarray(x, dtype=np.float32))
    f = np.float32
    bff = ml_dtypes.bfloat16
    rep4 = lambda v: np.tile(np.asarray(v, f).reshape(-1), 4)[:, None]
    woT = np.asarray(Wo, f).T.astype(bff)  # [S, C]
    woP = np.zeros((128, 2, 128), dtype=bff)
    for p in range(2):
        for half in range(2):
            r = 2 * p + half
            woP[32 * r:32 * r + 32, p, 64 * half:64 * half + 64] = woT
    wkqv = np.concatenate([np.asarray(w, f).T.astype(bff)
                           for w in (Wk, Wq, Wv)], axis=1)  # [C, 3S]
    brs = np.concatenate([rep4(bk), rep4(bq), rep4(bv)], axis=1)  # [128, 3]
    consts = {
        "wkqvT": np.ascontiguousarray(wkqv),
        "woPP": np.ascontiguousarray(woP.reshape(128, 256)),
        "brs": np.ascontiguousarray(brs.astype(f)),
    }
    in_maps = []
    for i in range(NCORES):
        m = {"x": np.ascontiguousarray(x[:, :, :, i * HS:(i + 1) * HS, :])}
        m.update(consts)
        in_maps.append(m)
    return in_maps


def gather(results, bo):
    out = np.empty((B, C, D, H, W), dtype=np.float32)
    for i in range(NCORES):
        out[:, :, :, i * HS:(i + 1) * HS, :] = results[i]["y"]
    out += np.asarray(bo, np.float32)[None, :, None, None, None]
    return out


def kernel(x, Wk, bk, Wq, bq, Wv, bv, Wo, bo):
    nc = get_nc()
    in_maps = make_in_maps(x, Wk, bk, Wq, bq, Wv, bv, Wo, bo)
    res = run_bass_kernel_spmd(nc, in_maps, core_ids=list(range(NCORES)))
    return gather(res.results, bo)
